# revision 2
# baseline (speedup 1.0000x reference)
"""DTVNet TV-prox cascade kernel for 8 Trainium2 NeuronCores (v2).

Design (hardcoded for image/sino [2, 256, 256, 128] f32):
  - Data-parallel along D (axis 1): core k owns D slabs [32k, 32k+32) and
    receives a 38-slab chunk (exact dependency halo of 3 per side,
    zero-padded at global edges). Per-cascade compute windows shrink by one
    slab/col per side, making every owned output exact with halo 3.
  - fp16 internals, [W=128 partitions, Dslab, Hcols, B=2] layout. B
    innermost keeps every D/H stencil shift 4-byte aligned, so DVE
    tensor_tensor runs in 2x mode and tensor_scalar (clamps) in 4x mode.
  - Precision (gate 2e-2, this lands ~1e-2): the host pre-scales the f32
    image by (1-lamb) and the t extraction bakes the (1-lamb) in (host
    descales outputs), so the loop state ts = (1-lamb)*t. Dual increments
    use the delta form w = dts + lamb*dsino (small-magnitude roundings
    only; cascade 0 differences the exact f32 image). The W-axis stencil
    runs as M1n@ts + M1n@ss. znew = clip(ts+ss) is fp16-safe since the
    clip at +-sigma kills large-magnitude rounding.
  - W-axis stencils and the whole t assembly run on the TensorEngine:
    identity/stencil matmuls of (shifted) p/q/znew/st accumulate in PSUM
    f32; ScalarE extracts PSUM->SBUF fp16. q ops run on GpSimd; clamps on
    DVE (fp16 4x tensor_scalar).
  - H processed in 5 chunks (halo 3) so state fits SBUF double-buffered
    for cross-chunk pipelining.
"""

import sys

import numpy as np

sys.path.insert(0, "/opt/trn_rl_repo")

_B, _D, _H, _W = 2, 256, 256, 128
_NCORES = 8
_DCH = _D // _NCORES          # 32 owned D slabs per core
_HD = 3                       # D halo (exact dependency radius)
_ND = _DCH + 2 * _HD          # 38 slabs incl ghosts
_OWNED_H = [52, 52, 52, 52, 48]
_HH = 3                       # H halo
_LAMB = 0.01
_CASC = 3
_PSUM_BANK = 512              # f32 elems per PSUM bank per partition

# engine toggles
_Q_ON_POOL = False            # q add/clamp on gpsimd (HW: ~12us/op overhead!)
_MASK_ON_POOL = False         # p edge masks on gpsimd
_SS_ON_ACT = True             # ss scale + t16 cast on ScalarE
# debug: skip blocks for differential timing (breaks correctness)
_SKIP_PE = False
_SKIP_DVEW = False
_SKIP_QP = False

_RUNNER_CACHE = {}


def _h_chunks():
    out = []
    oh = 0
    for i, og in enumerate(_OWNED_H):
        h0 = max(0, oh - _HH)
        h1 = min(_H, oh + og + _HH)
        out.append(
            dict(h0=h0, h1=h1, F=h1 - h0, ow0=oh - h0, oh=oh, og=og,
                 left=(i == 0), right=(i == len(_OWNED_H) - 1))
        )
        oh += og
    assert oh == _H
    return out


def _stencil_mats():
    # M1n: out[p] = z[p] - z[p+1] for p<127, 0 at p=127  ( = -fwd_diff_W )
    m1n = np.zeros((128, 128), np.float32)
    for p in range(127):
        m1n[p, p] = 1.0
        m1n[p + 1, p] = -1.0
    # M2p: adjoint: out[0] = -s[0]; out[p] = s[p-1]-s[p]; out[127] = s[126]
    m2p = np.zeros((128, 128), np.float32)
    for p in range(128):
        if p >= 1:
            m2p[p - 1, p] = 1.0
        if p <= 126:
            m2p[p, p] = -1.0
    ii = np.eye(128, dtype=np.float32)
    # [M1N, M2P, MI, MIN]
    return np.stack([m1n, m2p, ii, -ii]).astype(np.float16)


_M1N, _M2P, _MI, _MIN = range(4)


def _build_program(sigma, repeat=1):
    import contextlib

    from concourse import bacc, mybir
    from concourse.alu_op_type import AluOpType as OP
    from concourse.tile import TileContext

    f16 = mybir.dt.float16
    f32 = mybir.dt.float32
    s0, s1, s2, s3 = [float(x) for x in sigma]
    nc = bacc.Bacc()
    img32 = nc.declare_dram_parameter("img32", [_W, _ND, _H, _B], f32, isOutput=False)
    sin = nc.declare_dram_parameter("sino", [_W, _ND, _H, _B], f16, isOutput=False)
    mats = nc.declare_dram_parameter("mats", [4, 128, 128], f16, isOutput=False)
    maskp = nc.declare_dram_parameter("maskp", [128, _ND], f16, isOutput=False)
    outs = [
        nc.declare_dram_parameter(f"out{c}", [_W, _DCH, _H, _B], f16, isOutput=True)
        for c in range(_CASC)
    ]

    with TileContext(nc) as tc:
        with (
            tc.tile_pool(name="const", bufs=1) as cpool,
            tc.tile_pool(name="t32p", bufs=1) as t32pool,
            tc.tile_pool(name="st16", bufs=2) as spool,
            tc.tile_pool(name="ps", bufs=2, space="PSUM") as ppool,
        ):
            matsb = cpool.tile([128, 4, 128], f16)
            for mi in range(4):
                nc.sync.dma_start(out=matsb[:, mi, :], in_=mats[mi, :, :])
            msb = cpool.tile([128, _ND], f16)
            nc.sync.dma_start(out=msb[:], in_=maskp[:])

            rep_ctx = (
                tc.For_i(0, repeat, 1) if repeat > 1 else contextlib.nullcontext()
            )
            with rep_ctx:
              for ch in _h_chunks():
                F = ch["F"]
                t32 = t32pool.tile([128, _ND, F, _B], f32, tag="t32")
                nc.sync.dma_start(out=t32[:], in_=img32[:, :, ch["h0"]:ch["h1"], :])
                ss = spool.tile([128, _ND, F, _B], f16, tag="ss")
                nc.sync.dma_start(out=ss[:], in_=sin[:, :, ch["h0"]:ch["h1"], :])
                ts = spool.tile([128, _ND, F, _B], f16, tag="ts")
                nc.scalar.mul(ss[:], ss[:], _LAMB)
                nc.scalar.copy(out=ts[:], in_=t32[:])
                dssd = spool.tile([128, _ND, F, _B], f16, tag="dssd")
                dssh = spool.tile([128, _ND, F, _B], f16, tag="dssh")
                nc.vector.tensor_tensor(
                    dssd[:, 0:_ND - 1], ss[:, 0:_ND - 1], ss[:, 1:_ND], OP.subtract
                )
                nc.vector.tensor_tensor(
                    dssh[:, :, 0:F - 1], ss[:, :, 0:F - 1], ss[:, :, 1:F],
                    OP.subtract,
                )
                z = spool.tile([128, _ND, F, _B], f16, tag="z")
                p = spool.tile([128, _ND, F, _B], f16, tag="p")
                q = spool.tile([128, _ND, F, _B], f16, tag="q")
                st = spool.tile([128, _ND, F, _B], f16, tag="st")
                wd = spool.tile([128, _ND, F, _B], f16, tag="wd")
                wh = spool.tile([128, _ND, F, _B], f16, tag="wh")

                for c in range(_CASC):
                    r = c + 1
                    # windows: D slabs
                    td0, td1 = r, _ND - r            # t
                    dd0, dd1 = r - 1, _ND - r        # duals
                    # windows: H cols
                    lt = 0 if ch["left"] else r
                    rt = F if ch["right"] else F - r
                    qa = max(lt - 1, 0)
                    zh0, zh1 = qa, min(rt + 1, F)
                    wq_end = rt if not ch["right"] else F - 1
                    W1 = (zh1 - zh0) * _B
                    W2 = (rt - lt) * _B

                    tsrc = t32 if c == 0 else ts  # f32 image at cascade 0

                    # ---- w_p = (ts[d]-ts[d+1]) + dssd ----
                    nc.vector.tensor_tensor(
                        wd[:, dd0:dd1, zh0:zh1],
                        tsrc[:, dd0:dd1, zh0:zh1],
                        tsrc[:, dd0 + 1:dd1 + 1, zh0:zh1],
                        OP.subtract,
                    )
                    nc.vector.tensor_tensor(
                        wd[:, dd0:dd1, zh0:zh1],
                        wd[:, dd0:dd1, zh0:zh1],
                        dssd[:, dd0:dd1, zh0:zh1],
                        OP.add,
                    )
                    # ---- p update ----
                    if c == 0:
                        nc.vector.tensor_scalar(
                            p[:, dd0:dd1, zh0:zh1],
                            wd[:, dd0:dd1, zh0:zh1],
                            -s0, s0, OP.max, OP.min,
                        )
                    else:
                        nc.vector.tensor_tensor(
                            p[:, dd0:dd1, zh0:zh1],
                            p[:, dd0:dd1, zh0:zh1],
                            wd[:, dd0:dd1, zh0:zh1],
                            OP.add,
                        )
                        nc.vector.tensor_scalar(
                            p[:, dd0:dd1, zh0:zh1],
                            p[:, dd0:dd1, zh0:zh1],
                            -s0, s0, OP.max, OP.min,
                        )
                    # mask p at global D edges (per-core mask from DRAM)
                    for (e0, e1) in ((dd0, _HD), (_ND - _HD - 1, dd1)):
                        if e0 >= e1:
                            continue
                        n = e1 - e0
                        mb = (
                            msb[:, e0:e1]
                            .unsqueeze(2)
                            .unsqueeze(3)
                            .broadcast_to([128, n, zh1 - zh0, _B])
                        )
                        meng = nc.gpsimd if _MASK_ON_POOL else nc.vector
                        meng.tensor_tensor(
                            p[:, e0:e1, zh0:zh1], p[:, e0:e1, zh0:zh1],
                            mb, OP.mult,
                        )

                    # ---- w_q = (ts[h]-ts[h+1]) + dssh ----
                    nc.vector.tensor_tensor(
                        wh[:, dd0:dd1, qa:wq_end],
                        tsrc[:, dd0:dd1, qa:wq_end],
                        tsrc[:, dd0:dd1, qa + 1:wq_end + 1],
                        OP.subtract,
                    )
                    nc.vector.tensor_tensor(
                        wh[:, dd0:dd1, qa:wq_end],
                        wh[:, dd0:dd1, qa:wq_end],
                        dssh[:, dd0:dd1, qa:wq_end],
                        OP.add,
                    )
                    if ch["right"]:
                        nc.vector.memset(wh[:, dd0:dd1, F - 1:F], 0.0)
                    # ---- q update (gpsimd) ----
                    qeng = nc.gpsimd if _Q_ON_POOL else nc.vector
                    if c == 0:
                        qeng.tensor_scalar(
                            q[:, dd0:dd1, qa:rt],
                            wh[:, dd0:dd1, qa:rt],
                            -s1, s1, OP.max, OP.min,
                        )
                    else:
                        qeng.tensor_tensor(
                            q[:, dd0:dd1, qa:rt],
                            q[:, dd0:dd1, qa:rt],
                            wh[:, dd0:dd1, qa:rt],
                            OP.add,
                        )
                        qeng.tensor_scalar(
                            q[:, dd0:dd1, qa:rt],
                            q[:, dd0:dd1, qa:rt],
                            -s1, s1, OP.max, OP.min,
                        )

                    # ---- znew = clip(ts + ss) on t window ----
                    nc.vector.tensor_tensor(
                        z[:, td0:td1, lt:rt],
                        ts[:, td0:td1, lt:rt],
                        ss[:, td0:td1, lt:rt],
                        OP.add,
                    )
                    nc.vector.tensor_scalar(
                        z[:, td0:td1, lt:rt],
                        z[:, td0:td1, lt:rt],
                        -s3, s3, OP.max, OP.min,
                    )

                    # Multi-slab matmul chain: 4 slabs per PSUM bank, 4 banks
                    # per ps tile; term-major across banks (groups must stay
                    # contiguous within a bank). terms: (mat, rhs(a, b), lo)
                    # with lo>0 emitted per-slab (partial columns).
                    def emit_chain(terms, d0, d1, Wc, extract):
                        d = d0
                        while d < d1:
                            dn = min(d + 16, d1)
                            ps = ppool.tile([128, 4, _PSUM_BANK], f32, tag="ps")
                            nb = (dn - d + 3) // 4
                            for ti, (mi, rhs_fn, lo) in enumerate(terms):
                                for j in range(nb):
                                    b0 = d + 4 * j
                                    b1 = min(b0 + 4, dn)
                                    if lo:
                                        for sl in range(b0, b1):
                                            k = sl - b0
                                            nc.tensor.matmul(
                                                ps[:, j, k * Wc + lo:(k + 1) * Wc],
                                                matsb[:, mi, :],
                                                rhs_fn(sl, sl + 1),
                                                start=False, stop=False,
                                                skip_group_check=True,
                                            )
                                    else:
                                        nc.tensor.matmul(
                                            ps[:, j, 0:(b1 - b0) * Wc],
                                            matsb[:, mi, :],
                                            rhs_fn(b0, b1),
                                            start=(ti == 0),
                                            stop=(ti == len(terms) - 1),
                                            skip_group_check=True,
                                        )
                            nbf, rem = divmod(dn - d, 4)
                            if nbf:
                                extract(d, d + 4 * nbf, ps[:, 0:nbf, 0:4 * Wc])
                            if rem:
                                extract(d + 4 * nbf, dn, ps[:, nbf, 0:rem * Wc])
                            d = dn

                    # ---- st chain: st = clip(st + M1n@ts + M1n@ss) ----
                    st_terms = [
                        (_M1N, lambda a, b: ts[:, a:b, zh0:zh1, :], 0),
                        (_M1N, lambda a, b: ss[:, a:b, zh0:zh1, :], 0),
                    ]
                    if c > 0:
                        st_terms.append(
                            (_MI, lambda a, b: st[:, a:b, zh0:zh1, :], 0))

                    def extract_st(a, b, pspart):
                        nc.scalar.copy(
                            out=st[:, a:b, zh0:zh1, :], in_=pspart)

                    emit_chain(st_terms, dd0, dd1, W1, extract_st)
                    nc.vector.tensor_scalar(
                        st[:, dd0:dd1, zh0:zh1],
                        st[:, dd0:dd1, zh0:zh1],
                        -s2, s2, OP.max, OP.min,
                    )

                    # ---- t chain: M2p@st + znew + q[h-1] - q + p[d-1] - p ----
                    if lt == 0:
                        qsh_term = (_MI, lambda a, b: q[:, a:b, 0:rt - 1, :], _B)
                    else:
                        qsh_term = (
                            _MI, lambda a, b: q[:, a:b, lt - 1:rt - 1, :], 0)
                    t_terms = [
                        (_M2P, lambda a, b: st[:, a:b, lt:rt, :], 0),
                        (_MI, lambda a, b: z[:, a:b, lt:rt, :], 0),
                        qsh_term,
                        (_MI, lambda a, b: p[:, a - 1:b - 1, lt:rt, :], 0),
                        (_MIN, lambda a, b: q[:, a:b, lt:rt, :], 0),
                        (_MIN, lambda a, b: p[:, a:b, lt:rt, :], 0),
                    ]

                    def extract_t(a, b, pspart):
                        nc.scalar.mul(ts[:, a:b, lt:rt, :], pspart, 1.0 - _LAMB)

                    emit_chain(t_terms, td0, td1, W2, extract_t)

                    # ---- output DMA (owned slabs/cols; host descales) ----
                    nc.sync.dma_start(
                        out=outs[c][:, :, ch["oh"]:ch["oh"] + ch["og"], :],
                        in_=ts[:, _HD:_HD + _DCH, ch["ow0"]:ch["ow0"] + ch["og"], :],
                    )
    nc.compile()
    return nc


def _make_runner(nc, n_cores):
    """Reusable (cached-jit) runner, modeled on concourse.bass2jax."""
    import jax
    from jax.experimental.shard_map import shard_map
    from jax.sharding import Mesh, PartitionSpec

    from concourse import bass2jax, mybir

    bass2jax.install_neuronx_cc_hook()

    partition_name = (
        nc.partition_id_tensor.name if nc.partition_id_tensor else None
    )
    in_names, out_names, out_avals = [], [], []
    for alloc in nc.m.functions[0].allocations:
        if not isinstance(alloc, mybir.MemoryLocationSet):
            continue
        name = alloc.memorylocations[0].name
        if alloc.kind == "ExternalInput":
            if name != partition_name:
                in_names.append(name)
        elif alloc.kind == "ExternalOutput":
            shape = tuple(alloc.tensor_shape)
            dtype = mybir.dt.np(alloc.dtype)
            out_names.append(name)
            out_avals.append(jax.core.ShapedArray(shape, dtype))
    n_params = len(in_names)
    n_outs = len(out_avals)
    all_in_names = tuple(in_names + out_names + ([partition_name] if partition_name else []))
    donate = tuple(range(n_params, n_params + n_outs))

    def _body(*args):
        operands = list(args)
        if partition_name is not None:
            operands.append(bass2jax.partition_id_tensor())
        return tuple(
            bass2jax._bass_exec_p.bind(
                *operands,
                out_avals=tuple(out_avals),
                in_names=all_in_names,
                out_names=tuple(out_names),
                lowering_input_output_aliases=(),
                sim_require_finite=True,
                sim_require_nnan=True,
                nc=nc,
            )
        )

    devices = jax.devices()[:n_cores]
    assert len(devices) == n_cores
    mesh = Mesh(np.asarray(devices), ("core",))
    in_specs = (PartitionSpec("core"),) * (n_params + n_outs)
    out_specs = (PartitionSpec("core"),) * n_outs
    sharded = jax.jit(
        shard_map(
            _body, mesh=mesh, in_specs=in_specs, out_specs=out_specs, check_rep=False
        ),
        donate_argnums=donate,
        keep_unused=True,
    )

    def _concat_inputs(in_maps):
        per_core = [[np.asarray(m[name]) for name in in_names] for m in in_maps]
        return [
            np.concatenate([per_core[c][i] for c in range(n_cores)], axis=0)
            for i in range(n_params)
        ]

    def run(in_maps):
        concat_in = _concat_inputs(in_maps)
        concat_zeros = [
            np.zeros((n_cores * a.shape[0], *a.shape[1:]), a.dtype) for a in out_avals
        ]
        out_arrs = sharded(*concat_in, *concat_zeros)
        return [
            {
                name: np.asarray(out_arrs[i]).reshape(
                    n_cores, *out_avals[i].shape
                )[c]
                for i, name in enumerate(out_names)
            }
            for c in range(n_cores)
        ]

    def time_device(in_maps, reps=20):
        """Device-exec wall time, inputs pre-staged on device."""
        import time as _time

        sharded_nodonate = jax.jit(
            shard_map(
                _body,
                mesh=mesh,
                in_specs=in_specs,
                out_specs=out_specs,
                check_rep=False,
            ),
            keep_unused=True,
        )
        from jax.sharding import NamedSharding

        concat_in = _concat_inputs(in_maps)
        concat_zeros = [
            np.zeros((n_cores * a.shape[0], *a.shape[1:]), a.dtype) for a in out_avals
        ]
        shard = NamedSharding(mesh, PartitionSpec("core"))
        dev_in = [jax.device_put(x, shard) for x in concat_in]
        dev_zero = [jax.device_put(x, shard) for x in concat_zeros]
        out = sharded_nodonate(*dev_in, *dev_zero)  # warm + compile
        jax.block_until_ready(out)
        times = []
        for _ in range(reps):
            t0 = _time.perf_counter()
            out = sharded_nodonate(*dev_in, *dev_zero)
            jax.block_until_ready(out)
            times.append(_time.perf_counter() - t0)
        return times

    run.time_device = time_device
    return run


def _get_runner(sigma):
    key = tuple(float(x) for x in np.asarray(sigma).ravel())
    if key not in _RUNNER_CACHE:
        nc = _build_program(sigma)
        _RUNNER_CACHE[key] = _make_runner(nc, _NCORES)
    return _RUNNER_CACHE[key]


def _build_in_maps(image, sino):
    from concurrent.futures import ThreadPoolExecutor

    mats = _stencil_mats()

    def one_core(k):
        d0 = k * _DCH - _HD
        img_c = np.zeros((_W, _ND, _H, _B), np.float32)
        sino_c = np.zeros((_W, _ND, _H, _B), np.float16)
        lo, hi = max(0, d0), min(_D, d0 + _ND)
        img_c[:, lo - d0:hi - d0] = (
            image[:, lo:hi].transpose(3, 1, 2, 0) * np.float32(1.0 - _LAMB)
        )
        sino_c[:, lo - d0:hi - d0] = sino[:, lo:hi].transpose(3, 1, 2, 0)
        gd = d0 + np.arange(_ND)
        maskp = np.broadcast_to(
            ((gd >= 0) & (gd <= _D - 2)).astype(np.float16), (128, _ND)
        ).copy()
        return {"img32": img_c, "sino": sino_c, "mats": mats, "maskp": maskp}

    with ThreadPoolExecutor(max_workers=_NCORES) as ex:
        return list(ex.map(one_core, range(_NCORES)))


def _reference_numpy(image, sino, sigma, nt):
    """Slow exact fallback for unexpected inputs (e.g. nt != 0)."""
    def fwd_diff(v, ax):
        d = np.diff(v, axis=ax)
        pad = [(0, 0)] * v.ndim
        pad[ax] = (0, 1)
        return np.pad(d, pad)

    def fwd_diff_t(pp, ax):
        n = pp.shape[ax]
        pad_front = [(0, 0)] * pp.ndim
        pad_front[ax] = (1, 0)
        a = np.pad(pp, pad_front)
        a = np.take(a, range(n), axis=ax)
        pad_back = [(0, 0)] * pp.ndim
        pad_back[ax] = (0, 1)
        b = np.pad(np.take(pp, range(n - 1), axis=ax), pad_back)
        return a - b

    t = image.astype(np.float32)
    out = [t]
    p = np.zeros_like(t)
    q = np.zeros_like(t)
    s = np.zeros_like(t)
    for c in range(_CASC):
        z = t - np.float32(_LAMB) * (t - sino)
        pn = np.clip(p - fwd_diff(z, 1), -sigma[0], sigma[0])
        qn = np.clip(q - fwd_diff(z, 2), -sigma[1], sigma[1])
        sn = np.clip(s - fwd_diff(z, 3), -sigma[2], sigma[2])
        zn = np.clip(z, -sigma[3], sigma[3])
        p = pn + nt[c] * (pn - p)
        q = qn + nt[c] * (qn - q)
        s = sn + nt[c] * (sn - s)
        t = fwd_diff_t(p, 1) + fwd_diff_t(q, 2) + fwd_diff_t(s, 3) + zn
        out.append(t.astype(np.float32))
    return tuple(out)


def kernel(image, sino, sigma, nt):
    image = np.asarray(image, np.float32)
    sino = np.asarray(sino, np.float32)
    sigma = np.asarray(sigma, np.float32)
    nt = np.asarray(nt, np.float32)

    if (
        image.shape != (_B, _D, _H, _W)
        or sino.shape != (_B, _D, _H, _W)
        or np.any(nt != 0.0)
    ):
        return _reference_numpy(image, sino, sigma, nt)

    try:
        return _device_path(image, sino, sigma)
    except Exception:
        try:
            return _device_path(image, sino, sigma)  # retry: transient wedge
        except Exception:
            return _reference_numpy(image, sino, sigma, nt)


def _device_path(image, sino, sigma):
    runner = _get_runner(sigma)
    results = runner(_build_in_maps(image, sino))

    from concurrent.futures import ThreadPoolExecutor

    def gather(c):
        # per-core [W, DCH, H, B] fp16 -> concat d -> [B, D, H, W] f32
        cat = np.concatenate(
            [results[k][f"out{c}"] for k in range(_NCORES)], axis=1
        )
        out = np.ascontiguousarray(cat.transpose(3, 1, 2, 0)).astype(np.float32)
        out *= np.float32(1.0 / (1.0 - _LAMB))
        return out

    with ThreadPoolExecutor(max_workers=_CASC) as ex:
        full = list(ex.map(gather, range(_CASC)))
    return (image, full[0], full[1], full[2])
